# revision 8
# baseline (speedup 1.0000x reference)
"""Trainium2 Bass kernel for nn_BiAttention (sparse_attention).

Math: the reference's attention matrix is rank-1 plus a mask bias:
    att[b,l,m] = input_dot[b,l] + s[b,m],  s[m] = memory[m]@w_mem1 - 1e30*(1-mask[m])
Row softmax over m is invariant to the per-row constant input_dot[b,l], so
    weight_one[b,l,:] = softmax_m(s)            (same for every l)
    output_one[b,l,:] = v_b := softmax_m(s) @ (memory @ W_mem2.T + b_mem2)
Likewise max_m att[b,l,m] = input_dot[b,l] + const, so
    weight_two[b,0,:] = softmax_l(input_dot)
    output_two[b,0,:] = softmax_l(input_dot) @ inp2
The output [N, 4*Ld, d] row blocks are:
    [0:2048]    inp2 = input @ W_in2.T + b_in2
    [2048:4096] v_b broadcast
    [4096:6144] inp2 * v_b
    [6144:8192] (output_two * v_b) broadcast

Sharding: pure data parallel, one batch element per NeuronCore (8 cores).

Layout strategy: big operands are staged to bf16 on the host and
pre-transposed (inputT [d, l], W_in2T / W_mem2T [d, o]) so the device
does ZERO PE transposes — lhsT/rhs tiles stream straight from DRAM.
That halves HBM reads (~9.2MB vs 18.4MB/core) and leaves the kernel
bound by the 33.55MB of mandatory f32 output writes (HBM/NC ~358 GB/s
-> ~119us floor). input_dot rides the main matmul as an extra N=1
matmul per (tile, k) reusing the already-loaded stationary operand.
Row broadcasts (bias, w_mem1, v, u) are rank-1 PE matmuls with a ones
stationary instead of DMA broadcasts. Reads go on the ACT HWDGE ring,
writes on the SP ring, in FIFO orders chosen so neither ring ever
waits on compute once the pipe fills.
"""

import numpy as np
import ml_dtypes

import concourse.bass as bass
import concourse.tile as tile
from concourse import bacc, mybir
from concourse.bass_utils import run_bass_kernel_spmd

F32 = mybir.dt.float32
F32R = mybir.dt.float32r
BF16 = mybir.dt.bfloat16
AX = mybir.AxisListType
OP = mybir.AluOpType
EXP = mybir.ActivationFunctionType.Exp

P = 128
BSZ, LD, LM, HID = 8, 2048, 512, 1024
KT = HID // P          # 8 hidden-dim chunks
LT = LD // P           # 16 l tiles
MT = LM // P           # 4 memory tiles
NPAIR = LT // 2        # 8 write pairs
N_CORES = 8

_NC_CACHE = None


def _build_nc():
    nc = bacc.Bacc("TRN2", target_bir_lowering=False, num_devices=N_CORES)

    inpT_d = nc.dram_tensor("inputT", [HID, LD], BF16, kind="ExternalInput").ap()
    wi2T_d = nc.dram_tensor("W_in2T", [HID, HID], BF16, kind="ExternalInput").ap()
    wm2T_d = nc.dram_tensor("W_mem2T", [HID, HID], BF16, kind="ExternalInput").ap()
    mem_d = nc.dram_tensor("memory", [LM, HID], BF16, kind="ExternalInput").ap()
    wi1_d = nc.dram_tensor("w_in1c", [P, KT], BF16, kind="ExternalInput").ap()
    wm1_d = nc.dram_tensor("w_mem1", [1, HID], BF16, kind="ExternalInput").ap()
    mask_d = nc.dram_tensor("mask", [1, LM], F32, kind="ExternalInput").ap()
    bi2_d = nc.dram_tensor("b_in2", [1, HID], F32, kind="ExternalInput").ap()
    bm2_d = nc.dram_tensor("b_mem2", [1, HID], F32, kind="ExternalInput").ap()
    out_d = nc.dram_tensor("out", [4 * LD, HID], F32, kind="ExternalOutput").ap()

    with tile.TileContext(nc) as tc:
        with (
            tc.tile_pool(name="const", bufs=1) as cpool,
            tc.tile_pool(name="bc", bufs=1) as bcpool,
            tc.tile_pool(name="wT", bufs=1) as wpool,
            tc.tile_pool(name="inT", bufs=1) as ipool,
            tc.tile_pool(name="mem", bufs=1) as mempool,
            tc.tile_pool(name="rows", bufs=1) as rowpool,
            tc.tile_pool(name="small", bufs=1) as smallpool,
            tc.tile_pool(name="scr", bufs=2) as scrpool,
            tc.tile_pool(name="i2", bufs=4) as i2pool,
            tc.tile_pool(name="prod", bufs=2) as prodpool,
            tc.tile_pool(name="pout", bufs=4, space="PSUM") as poutpool,
            tc.tile_pool(name="pidot", bufs=1, space="PSUM") as pidpool,
            tc.tile_pool(name="psS", bufs=1, space="PSUM") as pspool,
            tc.tile_pool(name="psm", bufs=1, space="PSUM") as psmpool,
        ):
            # ---------------- constants ----------------
            ones_row = cpool.tile([1, P], F32, tag="onesr")
            nc.vector.memset(ones_row[:], 1.0)
            ones_col = cpool.tile([P, 1], F32, tag="onesc")
            nc.vector.memset(ones_col[:], 1.0)
            ones_col_bf = cpool.tile([P, 1], BF16, tag="onescb")
            nc.vector.memset(ones_col_bf[:], 1.0)
            ones_row_bf = cpool.tile([1, P], BF16, tag="onesrb")
            nc.vector.memset(ones_row_bf[:], 1.0)
            e_f32 = cpool.tile([P, LT], F32, tag="ef32")
            e_tile = cpool.tile([P, LT], F32R, tag="etile")

            # ---------------- small loads (ACT ring) ----------------
            wi1_col = smallpool.tile([P, KT], BF16, tag="wi1")
            nc.scalar.dma_start(wi1_col[:], wi1_d[:])
            mask_col = smallpool.tile([P, MT], F32, tag="mskc")
            nc.scalar.dma_start(mask_col[:], mask_d.rearrange("1 (o p) -> p o", p=P))
            wm1_row = rowpool.tile([1, HID], BF16, tag="wm1r")
            nc.scalar.dma_start(wm1_row[:], wm1_d[:])
            bi2_row = rowpool.tile([1, HID], F32, tag="bi2r")
            nc.scalar.dma_start(bi2_row[:], bi2_d[:])
            bm2_row = rowpool.tile([1, HID], F32, tag="bm2r")
            nc.scalar.dma_start(bm2_row[:], bm2_d[:])

            # ---------------- big loads (ACT ring) ----------------
            # HBM is the shared bottleneck from t=0, so only total bytes and
            # FIFO feasibility matter; order: W_in2T, inputT[0:4], memory,
            # W_mem2T, inputT[4:8].
            w2t = []
            for k in range(KT):
                w2t_k = wpool.tile([P, HID], BF16, tag=f"w2t{k}", name=f"w2t{k}")
                nc.scalar.dma_start(w2t_k[:], wi2T_d[k * P:(k + 1) * P, :])
                w2t.append(w2t_k)
            inpT = []
            for k in range(KT):
                inpT_k = ipool.tile([P, LD], BF16, tag=f"inT{k}", name=f"inT{k}")
                inpT.append(inpT_k)
            for k in range(4):
                nc.scalar.dma_start(inpT[k][:], inpT_d[k * P:(k + 1) * P, :])
            mem_t = mempool.tile([P, MT, HID], BF16, tag="memt")
            nc.scalar.dma_start(mem_t[:], mem_d.rearrange("(j p) d -> p j d", p=P))
            wm2t = []
            for k in range(KT):
                wm2t_k = wpool.tile([P, HID], BF16, tag=f"wm2t{k}", name=f"wm2t{k}")
                nc.scalar.dma_start(wm2t_k[:], wm2T_d[k * P:(k + 1) * P, :])
                wm2t.append(wm2t_k)
            for k in range(4, KT):
                nc.scalar.dma_start(inpT[k][:], inpT_d[k * P:(k + 1) * P, :])

            # ---------------- PE row broadcasts ----------------
            def rank1_bcast(row_ap, dst_dtype, name, bf=False):
                bc = bcpool.tile([P, HID], dst_dtype, tag=name, name=name)
                ones = ones_row_bf if bf else ones_row
                for h in range(2):
                    ps = poutpool.tile([P, 512], F32, tag="out", name=f"{name}{h}")
                    nc.tensor.matmul(
                        ps[:], ones[:], row_ap[:, h * 512:(h + 1) * 512],
                        start=True, stop=True,
                    )
                    nc.scalar.copy(bc[:, h * 512:(h + 1) * 512], ps[:])
                return bc

            bi2_bc = rank1_bcast(bi2_row, F32, "bi2bc")
            wm1_bc = rank1_bcast(wm1_row, BF16, "wm1bc", bf=True)

            # ---------------- v path ----------------
            # s_col[p, j] = memory[j*128+p] @ w_mem1 + mask bias  (DVE)
            s_col = smallpool.tile([P, MT], F32, tag="scol")
            for j in range(MT):
                scr = scrpool.tile([P, HID], BF16, tag="sscr", name=f"sscr{j}")
                nc.vector.tensor_mul(scr[:], mem_t[:, j, :], wm1_bc[:])
                nc.vector.tensor_reduce(s_col[:, j:j + 1], scr[:], AX.X, OP.add)
            msk = smallpool.tile([P, MT], F32, tag="msk")
            nc.vector.tensor_scalar(msk[:], mask_col[:], -1.0, 1e30,
                                    OP.add, OP.mult)
            nc.vector.tensor_add(s_col[:], s_col[:], msk[:])
            e_s = smallpool.tile([P, MT], BF16, tag="es")
            nc.scalar.activation(e_s[:], s_col[:], EXP)

            # P_un[d] = sum_m e_s[m] * memory[m, d]  and  Z_s  (PE)
            pun_sb = rowpool.tile([1, HID], F32, tag="punr")
            for h in range(2):
                pun_ps = psmpool.tile([1, 512], F32, tag="psm", name=f"pun{h}")
                for j in range(MT):
                    nc.tensor.matmul(
                        pun_ps[:], e_s[:, j:j + 1],
                        mem_t[:, j, h * 512:(h + 1) * 512],
                        start=(j == 0), stop=(j == MT - 1),
                        skip_group_check=True,
                    )
                nc.scalar.copy(pun_sb[:, h * 512:(h + 1) * 512], pun_ps[:])
            zs_ps = psmpool.tile([1, MT], F32, tag="psm", name="zs")
            nc.tensor.matmul(zs_ps[:], ones_col_bf[:], e_s[:],
                             start=True, stop=True)
            zs_row = smallpool.tile([1, 1], F32, tag="zs")
            nc.vector.tensor_reduce(zs_row[:], zs_ps[:], AX.X, OP.add)
            rzs = smallpool.tile([1, 1], F32, tag="rzs")
            nc.vector.reciprocal(rzs[:], zs_row[:])

            # p as bf16 column chunks [128, KT] (SWDGE cast DMAs)
            p_col = smallpool.tile([P, KT], BF16, tag="pcol")
            for k in range(KT):
                nc.gpsimd.dma_start(
                    p_col[:, k:k + 1], pun_sb[:, k * P:(k + 1) * P]
                )

            # v_unb[o] = sum_d p[d] * W_mem2T[d, o]  (PE matvecs)
            v_row = rowpool.tile([1, HID], F32, tag="vrow")
            for h in range(2):
                v_ps = psmpool.tile([1, 512], F32, tag="psm", name=f"vps{h}")
                for k in range(KT):
                    nc.tensor.matmul(
                        v_ps[:], p_col[:, k:k + 1],
                        wm2t[k][:, h * 512:(h + 1) * 512],
                        start=(k == 0), stop=(k == KT - 1),
                        skip_group_check=True,
                    )
                nc.scalar.copy(v_row[:, h * 512:(h + 1) * 512], v_ps[:])
            nc.vector.tensor_scalar(v_row[:], v_row[:], rzs[:], None, OP.mult)
            nc.vector.tensor_add(v_row[:], v_row[:], bm2_row[:])
            v_bc = rank1_bcast(v_row, F32, "vbc")

            # ---------------- persistent main-loop state ----------------
            idps = pidpool.tile([P, LT], F32, tag="idot")
            s_ps = [pspool.tile([1, 512], F32, tag=f"s{h}", name=f"s{h}")
                    for h in range(2)]
            pair_tiles = {}
            prod_seq = [0]
            vw_seq = [0]

            def emit_tile_mms(t, psos):
                """8x (LDW; MM h0 N=512; MM h1 N=512; MM idot N=1)."""
                half = t % 2
                for k in range(KT):
                    lhsT = inpT[k][:, t * P:(t + 1) * P]
                    for h in range(2):
                        nc.tensor.matmul(
                            psos[2 * half + h][:], lhsT,
                            w2t[k][:, h * 512:(h + 1) * 512],
                            start=(k == 0), stop=(k == KT - 1),
                            skip_group_check=True,
                        )
                    nc.tensor.matmul(
                        idps[:, t:t + 1], lhsT, wi1_col[:, k:k + 1],
                        start=(k == 0), stop=(k == KT - 1),
                        skip_group_check=True,
                    )

            def emit_tile_post(t, pair, psos):
                half = t % 2
                for h in range(2):
                    nc.vector.tensor_add(
                        pair[:, half * HID + h * 512:half * HID + (h + 1) * 512],
                        psos[2 * half + h][:],
                        bi2_bc[:, h * 512:(h + 1) * 512],
                    )
                nc.scalar.activation(e_f32[:, t:t + 1], idps[:, t:t + 1], EXP)
                nc.scalar.copy(e_tile[:, t:t + 1], e_f32[:, t:t + 1])

            def emit_pair_write(j, pair):
                nc.sync.dma_start(
                    out_d[j * 2 * P:(j + 1) * 2 * P, :]
                        .rearrange("(t p) d -> p t d", p=P),
                    pair.bitcast(F32).rearrange("p (t d) -> p t d", d=HID),
                )

            def emit_s(t):
                """u accumulation: s_ps[h] += e[t] * inp2[t, h-half]."""
                j, half = t // 2, t % 2
                pair = pair_tiles[j]
                for h in range(2):
                    nc.tensor.matmul(
                        s_ps[h][:], e_tile[:, t:t + 1],
                        pair[:, half * HID + h * 512:half * HID + (h + 1) * 512],
                        start=(t == 0), stop=(t == LT - 1),
                        skip_group_check=True,
                    )

            def emit_prod():
                j = prod_seq[0]
                prod_seq[0] += 1
                pair = pair_tiles.pop(j)
                prod_sb = prodpool.tile([P, 2 * HID], F32, tag="prod",
                                        name=f"pr{j}")
                for half in range(2):
                    nc.vector.tensor_mul(
                        prod_sb[:, half * HID:(half + 1) * HID],
                        pair.bitcast(F32)[:, half * HID:(half + 1) * HID],
                        v_bc[:],
                    )
                nc.sync.dma_start(
                    out_d[2 * LD + j * 2 * P:2 * LD + (j + 1) * 2 * P, :]
                        .rearrange("(t p) d -> p t d", p=P),
                    prod_sb.rearrange("p (t d) -> p t d", d=HID),
                )

            def emit_vwrite():
                h = vw_seq[0]
                vw_seq[0] += 1
                nc.sync.dma_start(
                    out_d[LD + h * LD // 2:LD + (h + 1) * LD // 2, :]
                        .rearrange("(t p) d -> p t d", p=P),
                    v_bc[:, None, :].to_broadcast([P, LT // 2, HID]),
                )

            # ---------------- main loop ----------------
            for t in range(LT):
                j, half = t // 2, t % 2
                if half == 0:
                    pair = i2pool.tile([P, 2 * HID], F32R, tag="i2",
                                       name=f"i2_{j}")
                    psos = [poutpool.tile([P, 512], F32, tag="out",
                                          name=f"ps{j}_{q}") for q in range(4)]
                    pair_tiles[j] = (pair, psos)
                else:
                    pair, psos = pair_tiles[j]

                emit_tile_mms(t, psos)
                emit_tile_post(t, pair, psos)

                if half == 1:
                    pair_tiles[j] = pair
                    emit_pair_write(j, pair)

                # s accumulation trails by 2 tiles
                if t - 2 >= 0:
                    emit_s(t - 2)

                # prod + v writes trail at pair granularity
                if half == 1:
                    if j >= 3:
                        emit_prod()
                    if j in (4, 6):
                        emit_vwrite()

            emit_s(LT - 2)
            emit_s(LT - 1)
            while prod_seq[0] < NPAIR:
                emit_prod()
            while vw_seq[0] < 2:
                emit_vwrite()

            # ---------------- tail: out2 (u) rows ----------------
            z_ps = psmpool.tile([1, LT], F32, tag="psm", name="zp")
            nc.tensor.matmul(z_ps[:], ones_col[:], e_f32[:],
                             start=True, stop=True)
            z_row = smallpool.tile([1, LT], F32, tag="zrow")
            nc.scalar.copy(z_row[:], z_ps[:])
            z_sb = smallpool.tile([1, 1], F32, tag="z")
            nc.vector.tensor_reduce(z_sb[:], z_row[:], AX.X, OP.add)
            rz = smallpool.tile([1, 1], F32, tag="rz")
            nc.vector.reciprocal(rz[:], z_sb[:])
            s_row = rowpool.tile([1, HID], F32, tag="srow")
            for h in range(2):
                nc.scalar.copy(s_row[:, h * 512:(h + 1) * 512], s_ps[h][:])
            u_row = rowpool.tile([1, HID], F32, tag="urow")
            nc.vector.tensor_scalar(u_row[:], s_row[:], rz[:], None, OP.mult)
            nc.vector.tensor_mul(u_row[:], u_row[:], v_row[:])
            u_bc = rank1_bcast(u_row, F32, "ubc")
            nc.sync.dma_start(
                out_d[3 * LD:4 * LD, :].rearrange("(t p) d -> p t d", p=P),
                u_bc[:, None, :].to_broadcast([P, LT, HID]),
            )

    nc.finalize()
    return nc


def _get_nc():
    global _NC_CACHE
    if _NC_CACHE is None:
        _NC_CACHE = _build_nc()
    return _NC_CACHE


def _prep_in_maps(inputs):
    bf = ml_dtypes.bfloat16
    inp = np.asarray(inputs["input"], dtype=np.float32)
    mem = np.asarray(inputs["memory"], dtype=np.float32)
    mask = np.asarray(inputs["mask"], dtype=np.float32)
    w_in1 = np.asarray(inputs["w_in1"], np.float32).reshape(HID)
    w_mem1 = np.asarray(inputs["w_mem1"], np.float32).reshape(1, HID)
    W_in2 = np.asarray(inputs["W_in2"], np.float32)
    b_in2 = np.asarray(inputs["b_in2"], np.float32).reshape(1, HID)
    W_mem2 = np.asarray(inputs["W_mem2"], np.float32)
    b_mem2 = np.asarray(inputs["b_mem2"], np.float32).reshape(1, HID)

    wi2T = np.ascontiguousarray(W_in2.T.astype(bf))
    wm2T = np.ascontiguousarray(W_mem2.T.astype(bf))
    wi1c = np.ascontiguousarray(w_in1.reshape(KT, P).T.astype(bf))
    wm1 = np.ascontiguousarray(w_mem1.astype(bf))

    in_maps = []
    for b in range(N_CORES):
        in_maps.append({
            "inputT": np.ascontiguousarray(inp[b].T.astype(bf)),
            "memory": np.ascontiguousarray(mem[b].astype(bf)),
            "mask": np.ascontiguousarray(mask[b].reshape(1, LM)),
            "w_in1c": wi1c,
            "w_mem1": wm1,
            "W_in2T": wi2T,
            "b_in2": b_in2,
            "W_mem2T": wm2T,
            "b_mem2": b_mem2,
        })
    return in_maps


def kernel(**inputs) -> np.ndarray:
    nc = _get_nc()
    in_maps = _prep_in_maps(inputs)
    res = run_bass_kernel_spmd(nc, in_maps, core_ids=list(range(N_CORES)))
    return np.stack([res.results[c]["out"] for c in range(N_CORES)], axis=0)


# revision 38
# speedup vs baseline: 6.5072x; 6.5072x over previous
"""Trainium2 Bass kernel for nn_BiAttention (sparse_attention).

Math: the reference's attention matrix is rank-1 plus a mask bias:
    att[b,l,m] = input_dot[b,l] + s[b,m],  s[m] = memory[m]@w_mem1 - 1e30*(1-mask[m])
Row softmax over m is invariant to the per-row constant input_dot[b,l], so
    weight_one[b,l,:] = softmax_m(s)            (same for every l)
    output_one[b,l,:] = v_b := softmax_m(s) @ (memory @ W_mem2.T + b_mem2)
Likewise max_m att[b,l,m] = input_dot[b,l] + const, so
    weight_two[b,0,:] = softmax_l(input_dot)
    output_two[b,0,:] = softmax_l(input_dot) @ inp2
The output [N, 4*Ld, d] row blocks are:
    [0:2048]    inp2 = input @ W_in2.T + b_in2
    [2048:4096] v_b broadcast
    [4096:6144] inp2 * v_b
    [6144:8192] (output_two * v_b) broadcast

Sharding: pure data parallel, one batch element per NeuronCore (8 cores).

Layout strategy: big operands are staged to bf16 on the host and
pre-transposed (inputT [d, l], W_in2T / W_mem2T [d, o]) so the device
does ZERO PE transposes — lhsT/rhs tiles stream straight from DRAM.
That halves HBM reads (~9.2MB vs 18.4MB/core) and leaves the kernel
bound by the 33.55MB of mandatory f32 output writes (HBM/NC ~358 GB/s
-> ~119us floor). input_dot rides the main matmul as an extra N=1
matmul per (tile, k) reusing the already-loaded stationary operand.
Row broadcasts (bias, w_mem1, v, u) are rank-1 PE matmuls at f32r/bf16
full rate (f32r operands are produced by DVE copies to satisfy the BIR
f32r-rounding rule).

Scheduling: engine sequencers issue their own DMAs serially, inline
with their compute queue — so ALL big DMA rides the sync(SP) ring
(whose sequencer does nothing else) and ACT keeps only activations and
PSUM->SBUF copies. SP FIFO order = readiness order: memory + W_mem2T
first (the whole v path completes ~15-30us while input streams), then
(w2t[k], inpT[k]) pairs with the tiny p_col DMA spliced between them,
then the 8MB v block (buffering the pipe while PE produces pairs),
then pair/prod writes at PE cadence, the u block last. Outputs are
written as 1MB two-tile pairs; inp2 tiles are kept in SBUF until both
their s-path matmul (2 tiles behind) and prod multiply (2 pairs
behind) consume them.
"""

import numpy as np
import ml_dtypes

import concourse.bass as bass
import concourse.tile as tile
from concourse import bacc, mybir
from concourse.bass_utils import run_bass_kernel_spmd

F32 = mybir.dt.float32
F32R = mybir.dt.float32r
BF16 = mybir.dt.bfloat16
AX = mybir.AxisListType
OP = mybir.AluOpType
EXP = mybir.ActivationFunctionType.Exp

P = 128
BSZ, LD, LM, HID = 8, 2048, 512, 1024
KT = HID // P          # 8 hidden-dim chunks
LT = LD // P           # 16 l tiles
MT = LM // P           # 4 memory tiles
NPAIR = LT // 2        # 8 write pairs
N_CORES = 8

_NC_CACHE = None


def _build_nc():
    nc = bacc.Bacc("TRN2", target_bir_lowering=False, num_devices=N_CORES)

    inpT_d = nc.dram_tensor("inputT", [HID, LD], BF16, kind="ExternalInput").ap()
    wi2T_d = nc.dram_tensor("W_in2T", [HID, HID], BF16, kind="ExternalInput").ap()
    wm2T_d = nc.dram_tensor("W_mem2T", [HID, HID], BF16, kind="ExternalInput").ap()
    mem_d = nc.dram_tensor("memory", [LM, HID], BF16, kind="ExternalInput").ap()
    wi1_d = nc.dram_tensor("w_in1c", [P, KT], BF16, kind="ExternalInput").ap()
    wm1_d = nc.dram_tensor("w_mem1", [1, HID], BF16, kind="ExternalInput").ap()
    mask_d = nc.dram_tensor("mask", [1, LM], F32, kind="ExternalInput").ap()
    bi2_d = nc.dram_tensor("b_in2", [1, HID], F32, kind="ExternalInput").ap()
    bm2_d = nc.dram_tensor("b_mem2", [1, HID], F32, kind="ExternalInput").ap()
    out_d = nc.dram_tensor("out", [4 * LD, HID], F32, kind="ExternalOutput").ap()

    with tile.TileContext(nc) as tc:
        with (
            tc.tile_pool(name="const", bufs=1) as cpool,
            tc.tile_pool(name="bc", bufs=1) as bcpool,
            tc.tile_pool(name="wT", bufs=1) as wpool,
            tc.tile_pool(name="inT", bufs=1) as ipool,
            tc.tile_pool(name="mem", bufs=1) as mempool,
            tc.tile_pool(name="rows", bufs=1) as rowpool,
            tc.tile_pool(name="small", bufs=1) as smallpool,
            tc.tile_pool(name="scr", bufs=2) as scrpool,
            tc.tile_pool(name="ecol", bufs=4) as epool,
            tc.tile_pool(name="i2", bufs=6) as i2pool,
            tc.tile_pool(name="prod", bufs=3) as prodpool,
            tc.tile_pool(name="pout", bufs=4, space="PSUM") as poutpool,
            tc.tile_pool(name="pidot", bufs=1, space="PSUM") as pidpool,
            tc.tile_pool(name="psS", bufs=1, space="PSUM") as pspool,
        ):
            psmpool = poutpool  # [1,512]-class psums ride the pout rotation
            # ---------------- constants ----------------
            ones_row = cpool.tile([1, P], F32, tag="onesr")
            nc.vector.memset(ones_row[:], 1.0)
            ones_row_r = cpool.tile([1, P], F32R, tag="onesrr")
            nc.vector.tensor_copy(ones_row_r[:], ones_row[:])
            ones_col_bf = cpool.tile([P, 1], BF16, tag="onescb")
            nc.vector.memset(ones_col_bf[:], 1.0)
            ones_col = cpool.tile([P, 1], F32, tag="onesc")
            nc.vector.memset(ones_col[:], 1.0)
            ones_row_bf = cpool.tile([1, P], BF16, tag="onesrb")
            nc.vector.memset(ones_row_bf[:], 1.0)
            e_f32 = cpool.tile([P, LT], F32, tag="ef32")

            # ---------------- small loads (ACT ring) ----------------
            wi1_col = smallpool.tile([P, KT], BF16, tag="wi1")
            nc.scalar.dma_start(wi1_col[:], wi1_d[:])
            mask_col = smallpool.tile([P, MT], F32, tag="mskc")
            nc.scalar.dma_start(mask_col[:], mask_d.rearrange("1 (o p) -> p o", p=P))
            wm1_row = rowpool.tile([1, HID], BF16, tag="wm1r")
            nc.scalar.dma_start(wm1_row[:], wm1_d[:])
            bi2_row = rowpool.tile([1, HID], F32, tag="bi2r")
            nc.scalar.dma_start(bi2_row[:], bi2_d[:])
            bm2_row = rowpool.tile([1, HID], F32, tag="bm2r")
            nc.scalar.dma_start(bm2_row[:], bm2_d[:])
            bi2_row_r = rowpool.tile([1, HID], F32R, tag="bi2rr")
            nc.vector.tensor_copy(bi2_row_r[:], bi2_row[:])

            # ---------------- big loads (single SP ring) ----------------
            # ALL big DMA rides the sync(SP) ring: the SP sequencer does
            # nothing else, so loads never head-of-line block ACT/DVE
            # compute. memory + W_mem2T go first so the v path can complete
            # while the input streams; the input loads are emitted further
            # down so the tiny p_col DMAs can sit between them in the FIFO.
            mem_t = mempool.tile([P, MT, HID], BF16, tag="memt")
            nc.sync.dma_start(mem_t[:], mem_d.rearrange("(j p) d -> p j d", p=P))
            wm2t = []
            for k in range(KT):
                wm2t_k = wpool.tile([P, HID], BF16, tag=f"wm2t{k}", name=f"wm2t{k}")
                nc.sync.dma_start(wm2t_k[:], wm2T_d[k * P:(k + 1) * P, :])
                wm2t.append(wm2t_k)

            def emit_w2_inpT_loads(ks):
                for k in ks:
                    w2t_k = wpool.tile([P, HID], BF16, tag=f"w2t{k}",
                                       name=f"w2t{k}")
                    nc.sync.dma_start(w2t_k[:], wi2T_d[k * P:(k + 1) * P, :])
                    w2t.append(w2t_k)
                    inpT_k = ipool.tile([P, LD], BF16, tag=f"inT{k}",
                                        name=f"inT{k}")
                    nc.sync.dma_start(inpT_k[:], inpT_d[k * P:(k + 1) * P, :])
                    inpT.append(inpT_k)

            w2t, inpT = [], []

            # ---------------- PE row broadcasts ----------------
            # f32r/bf16 operands -> full PE rate; copies off ACT when they
            # would head-of-line block the loop's exps (use DVE then).
            def rank1_bcast(row_ap, ones, dst_dtype, name, copy_eng):
                bc = bcpool.tile([P, HID], dst_dtype, tag=name, name=name)
                for h in range(2):
                    ps = poutpool.tile([P, 512], F32, tag="out", name=f"{name}{h}")
                    nc.tensor.matmul(
                        ps[:], ones[:], row_ap[:, h * 512:(h + 1) * 512],
                        start=True, stop=True,
                    )
                    if copy_eng == "act":
                        nc.scalar.copy(bc[:, h * 512:(h + 1) * 512], ps[:])
                    else:
                        nc.vector.tensor_copy(bc[:, h * 512:(h + 1) * 512],
                                              ps[:])
                return bc

            bi2_bc = rank1_bcast(bi2_row_r, ones_row_r, F32, "bi2bc", "act")
            wm1_bc = rank1_bcast(wm1_row, ones_row_bf, BF16, "wm1bc", "act")

            # ---------------- v path (spliced between early tiles) --------
            s_col = smallpool.tile([P, MT], F32, tag="scol")
            e_s = smallpool.tile([P, MT], BF16, tag="es")
            pun_sb = rowpool.tile([1, HID], F32, tag="punr")
            p_col = smallpool.tile([P, KT], BF16, tag="pcol")
            p_colf = smallpool.tile([P, KT], F32, tag="pcolf")
            rzs = smallpool.tile([1, 1], F32, tag="rzs")
            v_row = rowpool.tile([1, HID], F32R, tag="vrow")
            vbc_box = [None]

            def emit_v_scol():
                # s_col[p, j] = memory[j*128+p] @ w_mem1 + mask bias  (DVE)
                for j in range(MT):
                    scr = scrpool.tile([P, HID], BF16, tag="sscr",
                                       name=f"sscr{j}")
                    nc.vector.tensor_mul(scr[:], mem_t[:, j, :], wm1_bc[:])
                    nc.vector.tensor_reduce(s_col[:, j:j + 1], scr[:], AX.X,
                                            OP.add)
                msk = smallpool.tile([P, MT], F32, tag="msk")
                nc.vector.tensor_scalar(msk[:], mask_col[:], -1.0, 1e30,
                                        OP.add, OP.mult)
                nc.vector.tensor_add(s_col[:], s_col[:], msk[:])
                nc.scalar.activation(e_s[:], s_col[:], EXP)

            def emit_v_pun():
                # P_un[d] = sum_m e_s[m] * memory[m, d]  and  Z_s  (PE)
                for h in range(2):
                    pun_ps = psmpool.tile([1, 512], F32, tag="out",
                                          name=f"pun{h}")
                    for j in range(MT):
                        nc.tensor.matmul(
                            pun_ps[:], e_s[:, j:j + 1],
                            mem_t[:, j, h * 512:(h + 1) * 512],
                            start=(j == 0), stop=(j == MT - 1),
                            skip_group_check=True,
                        )
                    nc.vector.tensor_copy(pun_sb[:, h * 512:(h + 1) * 512],
                                          pun_ps[:])
                zs_ps = psmpool.tile([1, MT], F32, tag="out", name="zs")
                nc.tensor.matmul(zs_ps[:], ones_col_bf[:], e_s[:],
                                 start=True, stop=True)
                zs_row = smallpool.tile([1, 1], F32, tag="zs")
                nc.vector.tensor_reduce(zs_row[:], zs_ps[:], AX.X, OP.add)
                nc.vector.reciprocal(rzs[:], zs_row[:])
                # p as bf16 column chunks [128, KT]: SWDGE cast DMAs on the
                # idle Pool ring (HW-proven; strided single-DMA and K=1-matmul
                # variants both corrupt/wedge real HW despite passing CoreSim)
                for k in range(KT):
                    nc.gpsimd.dma_start(
                        p_col[:, k:k + 1], pun_sb[:, k * P:(k + 1) * P]
                    )

            def emit_v_mv():
                # v_unb[o] = sum_d p[d] * W_mem2T[d, o]  (PE matvecs)
                for h in range(2):
                    v_ps = psmpool.tile([1, 512], F32, tag="out",
                                        name=f"vps{h}")
                    for k in range(KT):
                        nc.tensor.matmul(
                            v_ps[:], p_col[:, k:k + 1],
                            wm2t[k][:, h * 512:(h + 1) * 512],
                            start=(k == 0), stop=(k == KT - 1),
                            skip_group_check=True,
                        )
                    nc.vector.tensor_copy(v_row[:, h * 512:(h + 1) * 512],
                                          v_ps[:])
                nc.vector.tensor_scalar(v_row[:], v_row[:], rzs[:], None,
                                        OP.mult)
                nc.vector.tensor_add(v_row[:], v_row[:], bm2_row[:])
                vbc_box[0] = rank1_bcast(v_row, ones_row_r, F32, "vbc", "act")

            # ---------------- persistent main-loop state ----------------
            # idot psums split by tile parity so the exp of tile t never
            # WAR-blocks the accumulation of tile t+1
            idps_ab = [pidpool.tile([P, LT // 2], F32, tag=f"id{a}",
                                    name=f"id{a}") for a in range(2)]
            s_ps = [pspool.tile([1, 512], F32, tag=f"s{h}", name=f"s{h}")
                    for h in range(2)]
            pair_tiles = {}
            ecols = {}
            prod_seq = [0]
            vw_seq = [0]

            def emit_tile_mms(t, psos):
                """8x (LDW; MM h0 N=512; MM h1 N=512; MM idot N=1)."""
                half = t % 2
                for k in range(KT):
                    lhsT = inpT[k][:, t * P:(t + 1) * P]
                    for h in range(2):
                        nc.tensor.matmul(
                            psos[2 * half + h][:], lhsT,
                            w2t[k][:, h * 512:(h + 1) * 512],
                            start=(k == 0), stop=(k == KT - 1),
                            skip_group_check=True,
                        )
                    nc.tensor.matmul(
                        idps_ab[t % 2][:, t // 2:t // 2 + 1], lhsT,
                        wi1_col[:, k:k + 1],
                        start=(k == 0), stop=(k == KT - 1),
                        skip_group_check=True,
                    )

            def emit_tile_post(t, pair, psos):
                half = t % 2
                for h in range(2):
                    nc.vector.tensor_add(
                        pair[:, half * HID + h * 512:half * HID + (h + 1) * 512],
                        psos[2 * half + h][:],
                        bi2_bc[:, h * 512:(h + 1) * 512],
                    )
                nc.scalar.activation(e_f32[:, t:t + 1],
                                     idps_ab[t % 2][:, t // 2:t // 2 + 1], EXP)
                # per-tile e column (own slot: no WAR chain on a shared tile)
                ec = epool.tile([P, 1], F32R, tag="ec", name=f"ec{t}")
                nc.scalar.copy(ec[:], e_f32[:, t:t + 1])
                ecols[t] = ec

            def emit_pair_write(j, pair):
                nc.sync.dma_start(
                    out_d[j * 2 * P:(j + 1) * 2 * P, :]
                        .rearrange("(t p) d -> p t d", p=P),
                    pair.bitcast(F32).rearrange("p (t d) -> p t d", d=HID),
                )

            def emit_s(t):
                """u accumulation: s_ps[h] += e[t] * inp2[t, h-half]."""
                j, half = t // 2, t % 2
                pair = pair_tiles[j]
                ec = ecols.pop(t)
                for h in range(2):
                    nc.tensor.matmul(
                        s_ps[h][:], ec[:],
                        pair[:, half * HID + h * 512:half * HID + (h + 1) * 512],
                        start=(t == 0), stop=(t == LT - 1),
                        skip_group_check=True,
                    )

            def emit_prod():
                j = prod_seq[0]
                prod_seq[0] += 1
                pair = pair_tiles.pop(j)
                v_bc = vbc_box[0]
                prod_sb = prodpool.tile([P, 2 * HID], F32, tag="prod",
                                        name=f"pr{j}")
                for half in range(2):
                    nc.vector.tensor_mul(
                        prod_sb[:, half * HID:(half + 1) * HID],
                        pair.bitcast(F32)[:, half * HID:(half + 1) * HID],
                        v_bc[:],
                    )
                nc.sync.dma_start(
                    out_d[2 * LD + j * 2 * P:2 * LD + (j + 1) * 2 * P, :]
                        .rearrange("(t p) d -> p t d", p=P),
                    prod_sb.rearrange("p (t d) -> p t d", d=HID),
                )

            def emit_vwrite():
                # 4MB v-block chunk, queued right after the reads: it buffers
                # the DMA pipe so pair/prod writes can backlog behind it while
                # PE produces them
                h = vw_seq[0]
                vw_seq[0] += 1
                v_bc = vbc_box[0]
                nc.sync.dma_start(
                    out_d[LD + h * LD // 2:LD + (h + 1) * LD // 2, :]
                        .rearrange("(t p) d -> p t d", p=P),
                    v_bc[:, None, :].to_broadcast([P, LT // 2, HID]),
                )

            # ---------------- v path + v writes, fully front-loaded ------
            # memory/W_mem2T land by ~11us; s_col/e_s/pun complete by ~15us
            # while the input streams. The tiny p_col DMAs (inside
            # emit_v_pun) sit between input-load pairs 2 and 3 in the SP
            # FIFO, where the ring arrives after their data is ready. The
            # 8MB v block then queues right after the reads and keeps the
            # DMA pipe dense while PE produces pairs.
            emit_v_scol()
            emit_w2_inpT_loads(range(0, 3))
            emit_v_pun()
            emit_w2_inpT_loads(range(3, KT))
            emit_v_mv()
            emit_vwrite()
            emit_vwrite()

            # ---------------- main loop ----------------
            for t in range(LT):
                j, half = t // 2, t % 2
                if half == 0:
                    pair = i2pool.tile([P, 2 * HID], F32R, tag="i2",
                                       name=f"i2_{j}")
                    psos = [poutpool.tile([P, 512], F32, tag="out",
                                          name=f"ps{j}_{q}") for q in range(4)]
                    pair_tiles[j] = (pair, psos)
                else:
                    pair, psos = pair_tiles[j]

                emit_tile_mms(t, psos)
                emit_tile_post(t, pair, psos)

                # s accumulation trails by 2 tiles
                if t - 2 >= 0:
                    emit_s(t - 2)

                if half == 1:
                    pair_tiles[j] = pair
                    emit_pair_write(j, pair)
                    if j >= 2:
                        emit_prod()

            emit_s(LT - 2)
            emit_s(LT - 1)
            while prod_seq[0] < NPAIR:
                emit_prod()

            # ---------------- tail: out2 (u) rows ----------------
            z_ps = psmpool.tile([1, LT], F32, tag="out", name="zp")
            nc.tensor.matmul(z_ps[:], ones_col[:], e_f32[:],
                             start=True, stop=True)
            z_sb = smallpool.tile([1, 1], F32, tag="z")
            nc.vector.tensor_reduce(z_sb[:], z_ps[:], AX.X, OP.add)
            rz = smallpool.tile([1, 1], F32, tag="rz")
            nc.vector.reciprocal(rz[:], z_sb[:])
            u_row = rowpool.tile([1, HID], F32R, tag="urow")
            for h in range(2):
                nc.vector.tensor_scalar(u_row[:, h * 512:(h + 1) * 512],
                                        s_ps[h][:], rz[:], None, OP.mult)
            nc.vector.tensor_mul(u_row[:], u_row[:], v_row.bitcast(F32)[:])
            u_bc = rank1_bcast(u_row, ones_row_r, F32, "ubc", "act")
            for h in range(2):
                nc.sync.dma_start(
                    out_d[3 * LD + h * LD // 2:3 * LD + (h + 1) * LD // 2, :]
                        .rearrange("(t p) d -> p t d", p=P),
                    u_bc[:, None, :].to_broadcast([P, LT // 2, HID]),
                )

    nc.finalize()
    return nc


def _get_nc():
    global _NC_CACHE
    if _NC_CACHE is None:
        _NC_CACHE = _build_nc()
    return _NC_CACHE


def _prep_in_maps(inputs):
    bf = ml_dtypes.bfloat16
    inp = np.asarray(inputs["input"], dtype=np.float32)
    mem = np.asarray(inputs["memory"], dtype=np.float32)
    mask = np.asarray(inputs["mask"], dtype=np.float32)
    w_in1 = np.asarray(inputs["w_in1"], np.float32).reshape(HID)
    w_mem1 = np.asarray(inputs["w_mem1"], np.float32).reshape(1, HID)
    W_in2 = np.asarray(inputs["W_in2"], np.float32)
    b_in2 = np.asarray(inputs["b_in2"], np.float32).reshape(1, HID)
    W_mem2 = np.asarray(inputs["W_mem2"], np.float32)
    b_mem2 = np.asarray(inputs["b_mem2"], np.float32).reshape(1, HID)

    wi2T = np.ascontiguousarray(W_in2.T.astype(bf))
    wm2T = np.ascontiguousarray(W_mem2.T.astype(bf))
    wi1c = np.ascontiguousarray(w_in1.reshape(KT, P).T.astype(bf))
    wm1 = np.ascontiguousarray(w_mem1.astype(bf))

    in_maps = []
    for b in range(N_CORES):
        in_maps.append({
            "inputT": np.ascontiguousarray(inp[b].T.astype(bf)),
            "memory": np.ascontiguousarray(mem[b].astype(bf)),
            "mask": np.ascontiguousarray(mask[b].reshape(1, LM)),
            "w_in1c": wi1c,
            "w_mem1": wm1,
            "W_in2T": wi2T,
            "b_in2": b_in2,
            "W_mem2T": wm2T,
            "b_mem2": b_mem2,
        })
    return in_maps


def kernel(**inputs) -> np.ndarray:
    nc = _get_nc()
    in_maps = _prep_in_maps(inputs)
    res = run_bass_kernel_spmd(nc, in_maps, core_ids=list(range(N_CORES)))
    return np.stack([res.results[c]["out"] for c in range(N_CORES)], axis=0)
